# revision 14
# baseline (speedup 1.0000x reference)
"""Attentional-GRU kernel for Trainium2 (8 NeuronCores, data-parallel).

Computes, for facts (B,S,H), G (B,S), weights Wr/Ur/W/U (H,H), biases:
    fWr = facts @ Wr.T + br ; fW = facts @ W.T + bw
    scan over t: r = sigmoid(fWr_t + C @ Ur.T + bur)
                 h~ = tanh(fW_t + r * (C @ U.T + bu))
                 C  = g_t * h~ + (1 - g_t) * C
returns final C (B, H).

Strategy: batch sharded over 8 cores (512 rows each). State C kept
*transposed* [h, b] on-chip so every matmul contracts h on the partition
axis. facts is pre-transposed on the host to [S, h, b] per shard; the
input projections are fused into the recurrence as PSUM accumulations
(r-gate) or copied through SBUF (w-gate), so facts is read exactly once.
Matmuls run in float32r (full PE rate, ~1e-4 relative error).
"""
import numpy as np
from contextlib import ExitStack

B, S, H = 4096, 64, 512
NCORES = 8
BS = B // NCORES          # batch rows per core
P = 128                   # partitions
KC = H // P               # contraction chunks
OC = H // P               # output-feature tiles

_cached_nc = None


def _build(n_steps=S, reps=1, hw_reps=1):
    """Build the per-core Bass kernel.

    reps > 1 unrolls the whole recurrence multiple times; hw_reps > 1
    wraps it in a hardware loop instead (no code-size growth). Both are
    timing aids; each repetition starts from C=0 because step 0 never
    reads the state.
    """
    import concourse.bass as bass
    import concourse.bacc as bacc
    import concourse.tile as tile
    from concourse import mybir

    f32 = mybir.dt.float32
    f32r = mybir.dt.float32r
    AF = mybir.ActivationFunctionType
    OP = mybir.AluOpType

    nc = bacc.Bacc("TRN2", target_bir_lowering=False, debug=False,
                   num_devices=NCORES)

    facts_d = nc.dram_tensor("facts_t", [n_steps, KC, P, BS], f32r,
                             kind="ExternalInput")
    gb_d = nc.dram_tensor("gb", [n_steps, P, BS], f32, kind="ExternalInput")
    w_names = ("wr_t", "ur_t", "w_t", "u_t")
    w_d = {n: nc.dram_tensor(n, [H, H], f32r, kind="ExternalInput")
           for n in w_names}
    b_names = ("bias_r", "bias_w", "bias_u")
    b_d = {n: nc.dram_tensor(n, [OC, P], f32, kind="ExternalInput")
           for n in b_names}
    out_d = nc.dram_tensor("out", [KC, P, BS], f32, kind="ExternalOutput")

    with tile.TileContext(nc) as tc, ExitStack() as ctx:
        PS = bass.MemorySpace.PSUM
        wpool = ctx.enter_context(tc.tile_pool(name="w", bufs=1))
        fring = ctx.enter_context(tc.tile_pool(name="facts", bufs=4))
        gring = ctx.enter_context(tc.tile_pool(name="g", bufs=4))
        cpool = ctx.enter_context(tc.tile_pool(name="c", bufs=2))
        tmp = ctx.enter_context(tc.tile_pool(name="tmp", bufs=2))
        w1pool = ctx.enter_context(tc.tile_pool(name="w1sb", bufs=8))
        psR = ctx.enter_context(tc.tile_pool(name="psR", bufs=4, space=PS))
        psW1 = ctx.enter_context(tc.tile_pool(name="psW1", bufs=2, space=PS))
        psW2 = ctx.enter_context(tc.tile_pool(name="psW2", bufs=2, space=PS))

        # load order matters at startup: wr_t/w_t feed the first projection
        # matmuls; ur_t/u_t are not needed until step 1 (~28 us in).
        wsb = {}
        for n in ("wr_t", "w_t", "ur_t", "u_t"):
            t = wpool.tile([P, KC, H], f32r, tag=n)
            nc.sync.dma_start(t[:], w_d[n].rearrange("(k p) o -> p k o", p=P))
            wsb[n] = t
        bsb = {}
        for n in b_names:
            t = wpool.tile([P, OC], f32, tag=n)
            nc.sync.dma_start(t[:], b_d[n].rearrange("k p -> p k"))
            bsb[n] = t

        PF = 2

        def one_pass(write_out):
            fts, gts = {}, {}

            def prefetch(t):
                if t < n_steps:
                    ft = fring.tile([P, KC, BS], f32r, tag="ft")
                    nc.sync.dma_start(ft[:], facts_d[t].rearrange("k p b -> p k b"))
                    gt = gring.tile([P, BS], f32, tag="gt")
                    nc.sync.dma_start(gt[:], gb_d[t])
                    fts[t], gts[t] = ft, gt

            def proj(t):
                """Emit input-projection matmuls for step t.

                r-gate projections open PSUM accumulation groups that the
                step-t recurrence matmuls will extend; w-gate projections
                are completed and copied to SBUF so their banks recycle.
                """
                ft = fts[t]
                Rs, W1s = [], []
                for ot in range(OC):
                    pr = psR.tile([P, BS], f32, tag="psR")
                    for k in range(KC):
                        nc.tensor.matmul(pr[:], wsb["wr_t"][:, k, ot * P:(ot + 1) * P],
                                         ft[:, k, :], start=(k == 0), stop=False,
                                         skip_group_check=True)
                    w1p = psW1.tile([P, BS], f32, tag="psW1")
                    for k in range(KC):
                        nc.tensor.matmul(w1p[:], wsb["w_t"][:, k, ot * P:(ot + 1) * P],
                                         ft[:, k, :], start=(k == 0), stop=(k == KC - 1),
                                         skip_group_check=True)
                    w1 = w1pool.tile([P, BS], f32, tag="w1sb")
                    nc.scalar.copy(w1[:], w1p[:])
                    Rs.append(pr)
                    W1s.append(w1)
                return Rs, W1s

            for t in range(PF + 1):
                prefetch(t)
            Rs, W1s = proj(0)
            C_prev = None
            for t in range(n_steps):
                prefetch(t + PF + 1)
                # C is stored as float32r (rounded on write by the producing
                # vector ops) so the recurrence matmuls can consume it.
                C_new = cpool.tile([P, KC, BS], f32r, tag="C")
                W2s = []
                if t > 0:
                    for ot in range(OC):
                        pr = Rs[ot]
                        for k in range(KC):
                            nc.tensor.matmul(pr[:], wsb["ur_t"][:, k, ot * P:(ot + 1) * P],
                                             C_prev[:, k, :],
                                             start=False, stop=(k == KC - 1),
                                             skip_group_check=True)
                        w2 = psW2.tile([P, BS], f32, tag="psW2")
                        for k in range(KC):
                            nc.tensor.matmul(w2[:], wsb["u_t"][:, k, ot * P:(ot + 1) * P],
                                             C_prev[:, k, :],
                                             start=(k == 0), stop=(k == KC - 1),
                                             skip_group_check=True)
                        W2s.append(w2)
                gt = gts[t]
                for ot in range(OC):
                    osl = (slice(None), slice(ot, ot + 1))
                    r = tmp.tile([P, BS], f32, tag="r")
                    nc.scalar.activation(r[:], Rs[ot][:], AF.Sigmoid,
                                         bias=bsb["bias_r"][osl])
                    s = tmp.tile([P, BS], f32, tag="s")
                    if t > 0:
                        m = tmp.tile([P, BS], f32, tag="m")
                        nc.vector.scalar_tensor_tensor(
                            m[:], W2s[ot][:], bsb["bias_u"][osl], r[:],
                            op0=OP.add, op1=OP.mult)
                        nc.vector.tensor_add(s[:], W1s[ot][:], m[:])
                    else:
                        # C0 == 0: h~ = tanh(fW + bw + r*bu)
                        nc.vector.scalar_tensor_tensor(
                            s[:], r[:], bsb["bias_u"][osl], W1s[ot][:],
                            op0=OP.mult, op1=OP.add)
                    ht = tmp.tile([P, BS], f32, tag="ht")
                    nc.scalar.activation(ht[:], s[:], AF.Tanh,
                                         bias=bsb["bias_w"][osl])
                    if t > 0:
                        cp = C_prev[:, ot, :].bitcast(f32)
                        # GPSIMD runs these ~3x slower than DVE, so give it
                        # only as many as it can hide under the matmul
                        # stream; the last o_tile (which gates the next
                        # step's matmuls) always stays on the DVE.
                        eng = nc.vector if ot in (0, OC - 1) else nc.gpsimd
                        d = tmp.tile([P, BS], f32, tag="d")
                        eng.tensor_sub(d[:], ht[:], cp)
                        e = tmp.tile([P, BS], f32, tag="e")
                        eng.tensor_mul(e[:], gt[:], d[:])
                        nc.vector.tensor_add(C_new[:, ot, :], cp, e[:])
                    else:
                        nc.vector.tensor_mul(C_new[:, ot, :], gt[:], ht[:])
                if t + 1 < n_steps:
                    Rs, W1s = proj(t + 1)
                C_prev = C_new

            if write_out:
                for k in range(KC):
                    nc.sync.dma_start(out_d[k], C_prev[:, k, :].bitcast(f32))

        if hw_reps > 1:
            assert reps == 1
            with tc.For_i(0, hw_reps, 1):
                one_pass(write_out=True)
        else:
            for rep in range(reps):
                one_pass(write_out=(rep == reps - 1))

    nc.compile()
    return nc


def _make_in_maps(facts, G, Wr, br, Ur, bur, W, bw, U, bu, n_steps=S):
    facts = np.asarray(facts, dtype=np.float32)
    G = np.asarray(G, dtype=np.float32)
    wr_t = np.ascontiguousarray(np.asarray(Wr, np.float32).T)
    ur_t = np.ascontiguousarray(np.asarray(Ur, np.float32).T)
    w_t = np.ascontiguousarray(np.asarray(W, np.float32).T)
    u_t = np.ascontiguousarray(np.asarray(U, np.float32).T)
    bias_r = np.ascontiguousarray(
        (np.asarray(br, np.float32) + np.asarray(bur, np.float32)).reshape(OC, P))
    bias_w = np.ascontiguousarray(np.asarray(bw, np.float32).reshape(OC, P))
    bias_u = np.ascontiguousarray(np.asarray(bu, np.float32).reshape(OC, P))

    in_maps = []
    for c in range(NCORES):
        sl = slice(c * BS, (c + 1) * BS)
        ft = np.ascontiguousarray(
            np.transpose(facts[sl, :n_steps], (1, 2, 0))).reshape(n_steps, KC, P, BS)
        gb = np.ascontiguousarray(
            np.broadcast_to(G[sl, :n_steps].T[:, None, :], (n_steps, P, BS)),
            dtype=np.float32)
        in_maps.append({
            "facts_t": ft, "gb": gb,
            "wr_t": wr_t, "ur_t": ur_t, "w_t": w_t, "u_t": u_t,
            "bias_r": bias_r, "bias_w": bias_w, "bias_u": bias_u,
        })
    return in_maps


LAST_RESULTS = None  # BassKernelResults of the most recent run (for profiling)


def kernel(facts, G, Wr, br, Ur, bur, W, bw, U, bu, _trace=False):
    global _cached_nc, LAST_RESULTS
    import os
    from concourse.bass_utils import run_bass_kernel_spmd

    if not _trace:
        # the axon client here has no NTFF hook; make sure an inherited
        # BASS_TRACE env var cannot push us onto that path
        os.environ["BASS_NEVER_TRACE"] = "1"

    if _cached_nc is None:
        _cached_nc = _build()
    in_maps = _make_in_maps(facts, G, Wr, br, Ur, bur, W, bw, U, bu)
    res = run_bass_kernel_spmd(_cached_nc, in_maps, list(range(NCORES)),
                               trace=_trace)
    LAST_RESULTS = res
    out = np.empty((B, H), dtype=np.float32)
    for c in range(NCORES):
        out[c * BS:(c + 1) * BS] = res.results[c]["out"].reshape(H, BS).T
    return out


# revision 16
# speedup vs baseline: 1.0837x; 1.0837x over previous
"""Attentional-GRU kernel for Trainium2 (8 NeuronCores, data-parallel).

Computes, for facts (B,S,H), G (B,S), weights Wr/Ur/W/U (H,H), biases:
    fWr = facts @ Wr.T + br ; fW = facts @ W.T + bw
    scan over t: r = sigmoid(fWr_t + C @ Ur.T + bur)
                 h~ = tanh(fW_t + r * (C @ U.T + bu))
                 C  = g_t * h~ + (1 - g_t) * C
returns final C (B, H).

Strategy: batch sharded over 8 cores (512 rows each). State C kept
*transposed* [h, b] on-chip so every matmul contracts h on the partition
axis. facts is pre-transposed on the host to [S, h, b] per shard; the
input projections are fused into the recurrence as PSUM accumulations
(r-gate) or copied through SBUF (w-gate), so facts is read exactly once.
Matmuls run in float32r (full PE rate, ~1e-4 relative error).
"""
import numpy as np
from contextlib import ExitStack

B, S, H = 4096, 64, 512
NCORES = 8
BS = B // NCORES          # batch rows per core
P = 128                   # partitions
KC = H // P               # contraction chunks
OC = H // P               # output-feature tiles

_cached_nc = None


def _build(n_steps=S, reps=1, hw_reps=1):
    """Build the per-core Bass kernel.

    reps > 1 unrolls the whole recurrence multiple times; hw_reps > 1
    wraps it in a hardware loop instead (no code-size growth). Both are
    timing aids; each repetition starts from C=0 because step 0 never
    reads the state.
    """
    import concourse.bass as bass
    import concourse.bacc as bacc
    import concourse.tile as tile
    from concourse import mybir

    f32 = mybir.dt.float32
    f32r = mybir.dt.float32r
    AF = mybir.ActivationFunctionType
    OP = mybir.AluOpType

    nc = bacc.Bacc("TRN2", target_bir_lowering=False, debug=False,
                   num_devices=NCORES)

    facts_d = nc.dram_tensor("facts_t", [n_steps, KC, P, BS], f32r,
                             kind="ExternalInput")
    gb_d = nc.dram_tensor("gb", [n_steps, P, BS], f32, kind="ExternalInput")
    w_names = ("wr_t", "ur_t", "w_t", "u_t")
    w_d = {n: nc.dram_tensor(n, [H, H], f32r, kind="ExternalInput")
           for n in w_names}
    b_names = ("bias_r", "bias_w", "bias_u")
    b_d = {n: nc.dram_tensor(n, [OC, P], f32, kind="ExternalInput")
           for n in b_names}
    out_d = nc.dram_tensor("out", [KC, P, BS], f32, kind="ExternalOutput")

    with tile.TileContext(nc) as tc, ExitStack() as ctx:
        PS = bass.MemorySpace.PSUM
        wpool = ctx.enter_context(tc.tile_pool(name="w", bufs=1))
        fring = ctx.enter_context(tc.tile_pool(name="facts", bufs=4))
        gring = ctx.enter_context(tc.tile_pool(name="g", bufs=4))
        cpool = ctx.enter_context(tc.tile_pool(name="c", bufs=2))
        tmp = ctx.enter_context(tc.tile_pool(name="tmp", bufs=2))
        w1pool = ctx.enter_context(tc.tile_pool(name="w1sb", bufs=8))
        psR = ctx.enter_context(tc.tile_pool(name="psR", bufs=4, space=PS))
        psW1 = ctx.enter_context(tc.tile_pool(name="psW1", bufs=2, space=PS))
        psW2 = ctx.enter_context(tc.tile_pool(name="psW2", bufs=2, space=PS))

        # load order matters at startup: wr_t/w_t feed the first projection
        # matmuls; ur_t/u_t are not needed until step 1 (~28 us in).
        wsb = {}
        for n in ("wr_t", "w_t", "ur_t", "u_t"):
            t = wpool.tile([P, KC, H], f32r, tag=n)
            nc.sync.dma_start(t[:], w_d[n].rearrange("(k p) o -> p k o", p=P))
            wsb[n] = t
        bsb = {}
        for n in b_names:
            t = wpool.tile([P, OC], f32, tag=n)
            nc.sync.dma_start(t[:], b_d[n].rearrange("k p -> p k"))
            bsb[n] = t

        PF = 2

        def one_pass(write_out):
            fts, gts = {}, {}

            def prefetch(t):
                if t < n_steps:
                    ft = fring.tile([P, KC, BS], f32r, tag="ft")
                    nc.sync.dma_start(ft[:], facts_d[t].rearrange("k p b -> p k b"))
                    gt = gring.tile([P, BS], f32, tag="gt")
                    nc.sync.dma_start(gt[:], gb_d[t])
                    fts[t], gts[t] = ft, gt

            def proj(t):
                """Emit input-projection matmuls for step t.

                r-gate projections open PSUM accumulation groups that the
                step-t recurrence matmuls will extend; w-gate projections
                are completed and copied to SBUF so their banks recycle.
                """
                ft = fts[t]
                Rs, W1s = [], []
                for ot in range(OC):
                    pr = psR.tile([P, BS], f32, tag="psR")
                    for k in range(KC):
                        nc.tensor.matmul(pr[:], wsb["wr_t"][:, k, ot * P:(ot + 1) * P],
                                         ft[:, k, :], start=(k == 0), stop=False,
                                         skip_group_check=True)
                    w1p = psW1.tile([P, BS], f32, tag="psW1")
                    for k in range(KC):
                        nc.tensor.matmul(w1p[:], wsb["w_t"][:, k, ot * P:(ot + 1) * P],
                                         ft[:, k, :], start=(k == 0), stop=(k == KC - 1),
                                         skip_group_check=True)
                    w1 = w1pool.tile([P, BS], f32, tag="w1sb")
                    nc.scalar.copy(w1[:], w1p[:])
                    Rs.append(pr)
                    W1s.append(w1)
                return Rs, W1s

            for t in range(PF + 1):
                prefetch(t)
            Rs, W1s = proj(0)
            C_prev = None
            for t in range(n_steps):
                prefetch(t + PF + 1)
                # C is stored as float32r (rounded on write by the producing
                # vector ops) so the recurrence matmuls can consume it.
                C_new = cpool.tile([P, KC, BS], f32r, tag="C")
                W2s = []
                if t > 0:
                    for ot in range(OC):
                        pr = Rs[ot]
                        for k in range(KC):
                            nc.tensor.matmul(pr[:], wsb["ur_t"][:, k, ot * P:(ot + 1) * P],
                                             C_prev[:, k, :],
                                             start=False, stop=(k == KC - 1),
                                             skip_group_check=True)
                        w2 = psW2.tile([P, BS], f32, tag="psW2")
                        for k in range(KC):
                            nc.tensor.matmul(w2[:], wsb["u_t"][:, k, ot * P:(ot + 1) * P],
                                             C_prev[:, k, :],
                                             start=(k == 0), stop=(k == KC - 1),
                                             skip_group_check=True)
                        W2s.append(w2)
                gt = gts[t]
                for ot in range(OC):
                    osl = (slice(None), slice(ot, ot + 1))
                    r = tmp.tile([P, BS], f32, tag="r")
                    nc.scalar.activation(r[:], Rs[ot][:], AF.Sigmoid,
                                         bias=bsb["bias_r"][osl])
                    s = tmp.tile([P, BS], f32, tag="s")
                    if t > 0:
                        m = tmp.tile([P, BS], f32, tag="m")
                        nc.vector.scalar_tensor_tensor(
                            m[:], W2s[ot][:], bsb["bias_u"][osl], r[:],
                            op0=OP.add, op1=OP.mult)
                        nc.vector.tensor_add(s[:], W1s[ot][:], m[:])
                    else:
                        # C0 == 0: h~ = tanh(fW + bw + r*bu)
                        nc.vector.scalar_tensor_tensor(
                            s[:], r[:], bsb["bias_u"][osl], W1s[ot][:],
                            op0=OP.mult, op1=OP.add)
                    ht = tmp.tile([P, BS], f32, tag="ht")
                    nc.scalar.activation(ht[:], s[:], AF.Tanh,
                                         bias=bsb["bias_w"][osl])
                    if t > 0:
                        cp = C_prev[:, ot, :].bitcast(f32)
                        # GPSIMD runs these ~3x slower than DVE, so give it
                        # only as many as it can hide under the matmul
                        # stream; the last o_tile (which gates the next
                        # step's matmuls) always stays on the DVE.
                        eng = nc.vector if ot in (0, OC - 1) else nc.gpsimd
                        d = tmp.tile([P, BS], f32, tag="d")
                        eng.tensor_sub(d[:], ht[:], cp)
                        e = tmp.tile([P, BS], f32, tag="e")
                        eng.tensor_mul(e[:], gt[:], d[:])
                        nc.vector.tensor_add(C_new[:, ot, :], cp, e[:])
                    else:
                        nc.vector.tensor_mul(C_new[:, ot, :], gt[:], ht[:])
                if t + 1 < n_steps:
                    Rs, W1s = proj(t + 1)
                C_prev = C_new

            if write_out:
                for k in range(KC):
                    nc.sync.dma_start(out_d[k], C_prev[:, k, :].bitcast(f32))

        if hw_reps > 1:
            assert reps == 1
            with tc.For_i(0, hw_reps, 1):
                one_pass(write_out=True)
        else:
            for rep in range(reps):
                one_pass(write_out=(rep == reps - 1))

    nc.compile()
    return nc


def _make_in_maps(facts, G, Wr, br, Ur, bur, W, bw, U, bu, n_steps=S):
    facts = np.asarray(facts, dtype=np.float32)
    G = np.asarray(G, dtype=np.float32)
    wr_t = np.ascontiguousarray(np.asarray(Wr, np.float32).T)
    ur_t = np.ascontiguousarray(np.asarray(Ur, np.float32).T)
    w_t = np.ascontiguousarray(np.asarray(W, np.float32).T)
    u_t = np.ascontiguousarray(np.asarray(U, np.float32).T)
    bias_r = np.ascontiguousarray(
        (np.asarray(br, np.float32) + np.asarray(bur, np.float32)).reshape(OC, P))
    bias_w = np.ascontiguousarray(np.asarray(bw, np.float32).reshape(OC, P))
    bias_u = np.ascontiguousarray(np.asarray(bu, np.float32).reshape(OC, P))

    in_maps = []
    for c in range(NCORES):
        sl = slice(c * BS, (c + 1) * BS)
        ft = np.ascontiguousarray(
            np.transpose(facts[sl, :n_steps], (1, 2, 0))).reshape(n_steps, KC, P, BS)
        gb = np.ascontiguousarray(
            np.broadcast_to(G[sl, :n_steps].T[:, None, :], (n_steps, P, BS)),
            dtype=np.float32)
        in_maps.append({
            "facts_t": ft, "gb": gb,
            "wr_t": wr_t, "ur_t": ur_t, "w_t": w_t, "u_t": u_t,
            "bias_r": bias_r, "bias_w": bias_w, "bias_u": bias_u,
        })
    return in_maps


LAST_RESULTS = None  # BassKernelResults of the most recent run (for profiling)


def kernel(facts, G, Wr, br, Ur, bur, W, bw, U, bu, _trace=False):
    global _cached_nc, LAST_RESULTS
    import os
    from concourse.bass_utils import run_bass_kernel_spmd

    if not _trace:
        # the axon client here has no NTFF hook; make sure an inherited
        # BASS_TRACE env var cannot push us onto that path
        os.environ["BASS_NEVER_TRACE"] = "1"

    if _cached_nc is None:
        _cached_nc = _build()
    in_maps = _make_in_maps(facts, G, Wr, br, Ur, bur, W, bw, U, bu)
    res = run_bass_kernel_spmd(_cached_nc, in_maps, list(range(NCORES)),
                               trace=_trace)
    LAST_RESULTS = res
    out = np.empty((B, H), dtype=np.float32)
    for c in range(NCORES):
        out[c * BS:(c + 1) * BS] = res.results[c]["out"].reshape(H, BS).T
    return out
